# revision 18
# baseline (speedup 1.0000x reference)
"""MoLoRA (top-2 MoE LoRA routing) Trainium2 kernel.

Full inputs -> shard tokens across 8 NeuronCores -> Bass/Tile kernel per core
-> gather full output.

Math (per token):
  logits = silu(x @ W1 + b1) @ W2 + b2
  top-2 softmax weights (renormalized over the top-2) == softmax over top-2
  logits; combined = sum_e w_e * (x @ A_e @ B_e) * 2.0 ; out = base + combined.

Kernel strategy per core (2048 tokens, 512-token tiles):
  - x is pre-transposed host-side to xT [D, TOK] (bf16) so every
    contraction over D streams straight from HBM into [D-part, token-free]
    tiles -- no on-chip transposes, no PSUM->SBUF copies. Loaded in
    quarter-slabs so the router can start before a full tile lands.
  - x/base/W1/A/B ship as bf16 (halves HBM traffic; verified rel err
    1.13e-2 vs the 2e-2 gate); the output is written bf16 on-device and
    upcast to f32 on the host at gather.
  - Router runs in token-on-free layout; logits are produced token-major
    directly (hs chunk stationary, W2 moving); top-2 softmax via max /
    masked-second-max / sigmoid-ratio / is_ge ops. Normalization (1/sum) is
    deferred into the epilogue.
  - Selected-expert weights are expanded to the stacked expert-rank dim [80]
    with a tiny 0/1 matmul, multiplied into lowT = A_all^T @ xT, and the
    combined output is lowscaled^T @ B_all (B pre-scaled by 2.0 on host),
    fused with  * (1/sum) + base_output  in one DVE op.
  - DMA queues are dedicated: xT on sync (qSP), base prefetch on scalar
    (qAct), out stores on gpsimd (SWDGE) so store waits never block loads.
    Finals for tile t interleave with router t+1 to keep the PE stream free
    of dependency stalls.
"""
import sys

for _p in ("/opt/trn_rl_repo",):
    if _p not in sys.path:
        sys.path.insert(0, _p)

import numpy as np
import ml_dtypes
from contextlib import ExitStack

import concourse.bass as bass
import concourse.tile as tile
from concourse import bacc, mybir
from concourse.bass_utils import run_bass_kernel_spmd

FP = mybir.dt.float32
FR = mybir.dt.float32r
BF = mybir.dt.bfloat16
NEG_BIG = -1e30

N_CORES = 8
B_, S, D = 4, 4096, 2048
E, R, H = 5, 16, 256
SCALING = 32.0 / 16.0
TT = 512
TOK = (B_ * S) // N_CORES


def _build_nc(TOK=TOK, D=D, H=H, E=E, R=R, TT=TT, n_cores=N_CORES):
    from concourse.alu_op_type import AluOpType as A

    NCH = TT // 128
    KD = D // 128
    KH = H // 128
    NT = TOK // TT
    M = E * R
    EP = 8
    ND = D // 512
    NQ = 4            # xT quarter-slabs per tile
    KQ = KD // NQ     # k-blocks per quarter

    assert TOK % TT == 0 and TT % 128 == 0 and D % 512 == 0 and H % 128 == 0

    nc = bacc.Bacc("TRN2", num_devices=n_cores, debug=False)

    xt_d = nc.dram_tensor("xt", [D, TOK], BF, kind="ExternalInput")
    base_d = nc.dram_tensor("base", [TOK, D], BF, kind="ExternalInput")
    a_d = nc.dram_tensor("a_all", [128, KD * M], BF, kind="ExternalInput")
    b_d = nc.dram_tensor("b_all", [M, D], BF, kind="ExternalInput")
    w1_d = nc.dram_tensor("w1", [128, KD * H], BF, kind="ExternalInput")
    b1_d = nc.dram_tensor("b1v", [128, KH], FP, kind="ExternalInput")
    w2_d = nc.dram_tensor("w2", [128, KH * EP], FR, kind="ExternalInput")
    b2b_d = nc.dram_tensor("b2b", [128, NCH * E], FP, kind="ExternalInput")
    e80_d = nc.dram_tensor("e80", [E, M], FR, kind="ExternalInput")
    id_d = nc.dram_tensor("ident", [128, 128], FR, kind="ExternalInput")
    out_d = nc.dram_tensor("out", [TOK, D], BF, kind="ExternalOutput")

    with tile.TileContext(nc) as tc, ExitStack() as ctx:
        const = ctx.enter_context(tc.tile_pool(name="const", bufs=1))
        xtq_pool = ctx.enter_context(tc.tile_pool(name="xtq", bufs=2 * NQ))
        base_pool = ctx.enter_context(tc.tile_pool(name="basep", bufs=8))
        out_pool = ctx.enter_context(tc.tile_pool(name="outp", bufs=6))
        hs_pool = ctx.enter_context(tc.tile_pool(name="hs", bufs=2))
        hst_pool = ctx.enter_context(tc.tile_pool(name="hst", bufs=2))
        sm_pool = ctx.enter_context(tc.tile_pool(name="sm", bufs=2))
        lsc_pool = ctx.enter_context(tc.tile_pool(name="lsc", bufs=2))

        ps_h = ctx.enter_context(tc.tile_pool(name="ps_h", bufs=3, space="PSUM"))
        ps_low = ctx.enter_context(tc.tile_pool(name="ps_low", bufs=1, space="PSUM"))
        ps_out = ctx.enter_context(tc.tile_pool(name="ps_out", bufs=4, space="PSUM"))

        ident = const.tile([128, 128], FR)
        nc.gpsimd.dma_start(ident[:], id_d.ap())
        # All weights at the head of the gpsimd (SWDGE) stream, before any
        # out-stores are queued there.
        w2_sb = const.tile([128, KH, EP], FR)
        nc.gpsimd.dma_start(w2_sb[:], w2_d.ap().rearrange("p (k e) -> p k e", e=EP))
        b1_sb = const.tile([128, KH], FP)
        nc.gpsimd.dma_start(b1_sb[:], b1_d.ap())
        b2b_sb = const.tile([128, NCH, E], FP)
        nc.gpsimd.dma_start(b2b_sb[:], b2b_d.ap().rearrange("p (c e) -> p c e", e=E))
        e80_sb = const.tile([E, M], FR)
        nc.gpsimd.dma_start(e80_sb[:], e80_d.ap())
        w1_sb = const.tile([128, KD, H], BF)
        a_sb = const.tile([128, KD, M], BF)
        bb_sb = const.tile([M, D], BF)
        nc.scalar.dma_start(w1_sb[:], w1_d.ap().rearrange("p (k h) -> p k h", h=H))
        nc.scalar.dma_start(a_sb[:], a_d.ap().rearrange("p (k m) -> p k m", m=M))
        nc.scalar.dma_start(bb_sb[:], b_d.ap())

        def load_xt(t):
            """Load tile t's xT as NQ quarter-slabs [128, KQ, TT]."""
            qs = []
            for q in range(NQ):
                xq = xtq_pool.tile([128, KQ, TT], BF, name="xq")
                nc.sync.dma_start(
                    xq[:],
                    xt_d.ap()[
                        q * KQ * 128 : (q + 1) * KQ * 128,
                        t * TT : (t + 1) * TT,
                    ].rearrange("(k p) n -> p k n", p=128),
                )
                qs.append(xq)
            return qs

        def load_base_chunk(t, c):
            tok0 = t * TT + c * 128
            base_sb = base_pool.tile([128, D], BF, name="base_sb")
            nc.scalar.dma_start(base_sb[:], base_d.ap()[tok0 : tok0 + 128, :])
            return base_sb

        def xt_k(qs, k):
            return qs[k // KQ][:, k % KQ, :]

        def emit_mm1(t, qs):
            h_ps = [
                ps_h.tile([128, 512], FP, tag="hps", name=f"h_ps{h}")
                for h in range(KH)
            ]
            for k in range(KD):
                for h in range(KH):
                    nc.tensor.matmul(
                        h_ps[h][:, 0:TT],
                        w1_sb[:, k, h * 128 : (h + 1) * 128],
                        xt_k(qs, k),
                        start=(k == 0),
                        stop=(k == KD - 1),
                    )
            return h_ps

        def emit_silu(t, h_ps):
            sg_sb = hst_pool.tile([128, KH, TT], FP)
            hs_sb = hs_pool.tile([128, KH, TT], FR)
            for h in range(KH):
                nc.vector.tensor_scalar(
                    hs_sb[:, h, :], h_ps[h][:, 0:TT], b1_sb[:, h : h + 1], None,
                    op0=A.add,
                )
                nc.scalar.activation(
                    sg_sb[:, h, :], h_ps[h][:, 0:TT],
                    mybir.ActivationFunctionType.Sigmoid,
                    bias=b1_sb[:, h : h + 1], scale=1.0,
                )
            nc.vector.tensor_tensor(hs_sb[:], hs_sb[:], sg_sb[:], A.mult)
            return hs_sb

        def emit_low(t, qs):
            low_ps = ps_low.tile([M, 512], FP)
            for k in range(KD):
                nc.tensor.matmul(
                    low_ps[:, 0:TT],
                    a_sb[:, k, :],
                    xt_k(qs, k),
                    start=(k == 0),
                    stop=(k == KD - 1),
                )
            return low_ps

        def emit_mm2_lg(t, hs_sb):
            # token-major logits directly: per chunk, hs slice is the
            # stationary operand, W2 streams (N=8) -- no copy/transpose hops
            lg_ps = ps_h.tile([128, 4, 128], FP, tag="hps")
            for c in range(NCH):
                for k in range(KH):
                    nc.tensor.matmul(
                        lg_ps[:, c, 0:EP],
                        hs_sb[:, k, c * 128 : (c + 1) * 128],
                        w2_sb[:, k, :],
                        start=(k == 0),
                        stop=(k == KH - 1),
                    )
            return lg_ps

        def emit_softmax(t, lg_ps):
            # top-2 softmax, unnormalized (1/sum fused into epilogue)
            Ls = sm_pool.tile([128, NCH, E], FP)
            nc.vector.tensor_tensor(Ls[:], lg_ps[:, 0:NCH, 0:E], b2b_sb[:], A.add)
            nm1 = sm_pool.tile([128, NCH], FP)
            nc.vector.tensor_reduce(
                nm1[:], Ls[:], axis=mybir.AxisListType.X, op=A.max, negate=True
            )
            mk = sm_pool.tile([128, NCH, E], FP)
            eq = sm_pool.tile([128, NCH, E], FP)
            for c in range(NCH):
                nc.vector.tensor_scalar(
                    eq[:, c, :], Ls[:, c, :], nm1[:, c : c + 1], 0.0,
                    op0=A.add, op1=A.is_equal,
                )
                nc.vector.scalar_tensor_tensor(
                    mk[:, c, :], eq[:, c, :], NEG_BIG, Ls[:, c, :],
                    op0=A.mult, op1=A.add,
                )
            nm2 = sm_pool.tile([128, NCH], FP)
            nc.vector.tensor_reduce(
                nm2[:], mk[:], axis=mybir.AxisListType.X, op=A.max, negate=True
            )
            vs = sm_pool.tile([128, NCH, E], FP)
            ve = sm_pool.tile([128, NCH, E], FP)
            om = sm_pool.tile([128, NCH, E], FP)
            ge = sm_pool.tile([128, NCH, E], FP)
            for c in range(NCH):
                nc.scalar.activation(
                    vs[:, c, :], Ls[:, c, :],
                    mybir.ActivationFunctionType.Sigmoid,
                    bias=nm1[:, c : c + 1], scale=1.0,
                )
                nc.vector.tensor_scalar(
                    ge[:, c, :], Ls[:, c, :], nm2[:, c : c + 1], 0.0,
                    op0=A.add, op1=A.is_ge,
                )
            nc.vector.tensor_scalar(
                om[:], vs[:], -1.0, 1.0, op0=A.mult, op1=A.add
            )
            nc.vector.reciprocal(om[:], om[:])
            nc.vector.tensor_tensor(ve[:], vs[:], om[:], A.mult)
            v = sm_pool.tile([128, NCH, E], FR)
            nc.vector.tensor_tensor(v[:], ve[:], ge[:], A.mult)
            s = sm_pool.tile([128, NCH], FP)
            nc.vector.tensor_reduce(s[:], v[:], axis=mybir.AxisListType.X, op=A.add)
            rinv = sm_pool.tile([128, NCH], FP)
            nc.vector.reciprocal(rinv[:], s[:])
            return v, rinv

        def emit_vt_we(t, v):
            vt_ps = ps_h.tile([EP, 512], FR, tag="hps")
            for c in range(NCH):
                nc.tensor.transpose(
                    vt_ps[0:E, c * 128 : (c + 1) * 128], v[:, c, :], ident[:]
                )
            vt_sb = sm_pool.tile([E, TT], FR)
            nc.scalar.copy(vt_sb[:], vt_ps[0:E, 0:TT])
            we_ps = ps_h.tile([M, 512], FP, tag="hps")
            nc.tensor.matmul(
                we_ps[:, 0:TT], e80_sb[:], vt_sb[:], start=True, stop=True
            )
            we_sb = lsc_pool.tile([M, TT], FP)
            nc.scalar.copy(we_sb[:], we_ps[:, 0:TT])
            return we_sb

        def emit_lsc(t, low_ps, we_sb):
            lsc_sb = lsc_pool.tile([M, TT], BF)
            nc.vector.tensor_tensor(lsc_sb[:], low_ps[:, 0:TT], we_sb[:], A.mult)
            return lsc_sb

        def emit_final_chunk(t, c, lsc_sb, rinv, base_sb):
            tok0 = t * TT + c * 128
            o_sb = out_pool.tile([128, D], BF)
            o_pss = []
            for db in range(ND):
                o_ps = ps_out.tile([128, 512], FP, tag="ops", name="o_ps")
                nc.tensor.matmul(
                    o_ps[:],
                    lsc_sb[:, c * 128 : (c + 1) * 128],
                    bb_sb[:, db * 512 : (db + 1) * 512],
                    start=True, stop=True,
                )
                o_pss.append(o_ps)
            for db in range(ND):
                nc.vector.scalar_tensor_tensor(
                    o_sb[:, db * 512 : (db + 1) * 512],
                    o_pss[db][:],
                    rinv[:, c : c + 1],
                    base_sb[:, db * 512 : (db + 1) * 512],
                    op0=A.mult, op1=A.add,
                )
            nc.gpsimd.dma_start(out_d.ap()[tok0 : tok0 + 128, :], o_sb[:])

        # ---- pipeline ----
        xts = {}     # t -> quarter tiles
        bases = {}   # t -> [base chunk tiles]
        lscs = {}    # t -> (lsc_sb, rinv)

        def emit_router(t):
            h_ps = emit_mm1(t, xts[t])
            low_ps = emit_low(t, xts[t])
            hs_sb = emit_silu(t, h_ps)
            lg_ps = emit_mm2_lg(t, hs_sb)
            v, rinv = emit_softmax(t, lg_ps)
            we_sb = emit_vt_we(t, v)
            lscs[t] = (emit_lsc(t, low_ps, we_sb), rinv)
            del xts[t]

        # prologue
        xts[0] = load_xt(0)
        bases[0] = [load_base_chunk(0, c) for c in range(NCH)]
        xts[1] = load_xt(1)
        emit_router(0)
        bases[1] = [load_base_chunk(1, c) for c in range(NCH)]

        for i in range(NT):
            r = i + 1
            if i + 2 < NT:
                xts[i + 2] = load_xt(i + 2)
            if i + 2 < NT:
                bases[i + 2] = [load_base_chunk(i + 2, c) for c in range(NCH)]
            lsc_sb, rinv = lscs.pop(i)
            b_tiles = bases.pop(i)
            # finals(i) interleaved with router(r): out-mm chunks 0,1 first,
            # then the big mm1 stream, then chunks 2,3, then the rest of the
            # router whose PE pieces are dependency-gated.
            emit_final_chunk(i, 0, lsc_sb, rinv, b_tiles[0])
            emit_final_chunk(i, 1, lsc_sb, rinv, b_tiles[1])
            if r < NT:
                h_ps = emit_mm1(r, xts[r])
                hs_sb = emit_silu(r, h_ps)
            emit_final_chunk(i, 2, lsc_sb, rinv, b_tiles[2])
            if r < NT:
                low_ps = emit_low(r, xts[r])
                lg_ps = emit_mm2_lg(r, hs_sb)
            emit_final_chunk(i, 3, lsc_sb, rinv, b_tiles[3])
            if r < NT:
                v, rinv2 = emit_softmax(r, lg_ps)
                we_sb = emit_vt_we(r, v)
                lscs[r] = (emit_lsc(r, low_ps, we_sb), rinv2)
                del xts[r]

    nc.compile()
    return nc


def _host_prep(x, base_output, A, B, W1, b1, W2, b2, n_cores=N_CORES, TT=TT,
               scaling=SCALING):
    Bb, S_, Dd = x.shape
    E_, _, R_ = A.shape
    N = Bb * S_
    TOKc = N // n_cores
    NCH = TT // 128
    xf = np.ascontiguousarray(x.reshape(N, Dd), dtype=np.float32)
    bf = np.ascontiguousarray(base_output.reshape(N, Dd), dtype=np.float32)
    a_all = A.transpose(1, 0, 2).reshape(Dd, E_ * R_)
    a_all = np.ascontiguousarray(
        a_all.reshape(Dd // 128, 128, E_ * R_).transpose(1, 0, 2).reshape(128, -1),
        np.float32)
    b_all = np.ascontiguousarray(B.reshape(E_ * R_, Dd) * scaling, np.float32)
    b2b = np.ascontiguousarray(
        np.broadcast_to(np.tile(np.asarray(b2, np.float32), NCH)[None, :],
                        (128, NCH * E_))
    )
    e80 = np.zeros((E_, E_ * R_), np.float32)
    for e in range(E_):
        e80[e, e * R_ : (e + 1) * R_] = 1.0
    ident = np.eye(128, dtype=np.float32)
    shared = {
        "a_all": a_all.astype(ml_dtypes.bfloat16),
        "b_all": b_all.astype(ml_dtypes.bfloat16),
        "w1": np.ascontiguousarray(
            np.asarray(W1, np.float32).reshape(Dd // 128, 128, -1)
            .transpose(1, 0, 2).reshape(128, -1)).astype(ml_dtypes.bfloat16),
        "b1v": np.ascontiguousarray(
            np.asarray(b1, np.float32).reshape(-1, 128).T),
        "w2": np.ascontiguousarray(
            np.pad(np.asarray(W2, np.float32), ((0, 0), (0, 8 - W2.shape[1])))
            .reshape(-1, 128, 8).transpose(1, 0, 2).reshape(128, -1)),
        "b2b": b2b,
        "e80": e80,
        "ident": ident,
    }
    in_maps = []
    for i in range(n_cores):
        m = dict(shared)
        m["xt"] = np.ascontiguousarray(
            xf[i * TOKc : (i + 1) * TOKc].T.astype(ml_dtypes.bfloat16))
        m["base"] = np.ascontiguousarray(
            bf[i * TOKc : (i + 1) * TOKc].astype(ml_dtypes.bfloat16))
        in_maps.append(m)
    return in_maps, (N, TOKc, Dd)


_NC_CACHE = {}


def _get_nc():
    if "nc" not in _NC_CACHE:
        _NC_CACHE["nc"] = _build_nc()
    return _NC_CACHE["nc"]


def kernel(x, base_output, A, B, W1, b1, W2, b2, _trace=False):
    x = np.asarray(x)
    base_output = np.asarray(base_output)
    nc = _get_nc()
    in_maps, (N, TOKc, Dd) = _host_prep(
        np.asarray(x, np.float32), np.asarray(base_output, np.float32),
        np.asarray(A, np.float32), np.asarray(B, np.float32),
        np.asarray(W1, np.float32), np.asarray(b1, np.float32),
        np.asarray(W2, np.float32), np.asarray(b2, np.float32),
    )
    res = run_bass_kernel_spmd(
        nc, in_maps, core_ids=list(range(N_CORES)), trace=_trace
    )
    out = np.concatenate(
        [np.asarray(res.results[i]["out"]).astype(np.float32) for i in range(N_CORES)],
        axis=0)
    out = out.reshape(x.shape)
    if _trace:
        kernel._last_exec_time_ns = res.exec_time_ns
        kernel._last_results = res
    return out


# revision 19
# speedup vs baseline: 1.0121x; 1.0121x over previous
"""MoLoRA (top-2 MoE LoRA routing) Trainium2 kernel.

Full inputs -> shard tokens across 8 NeuronCores -> Bass/Tile kernel per core
-> gather full output.

Math (per token):
  logits = silu(x @ W1 + b1) @ W2 + b2
  top-2 softmax weights (renormalized over the top-2) == softmax over top-2
  logits; combined = sum_e w_e * (x @ A_e @ B_e) * 2.0 ; out = base + combined.

Kernel strategy per core (2048 tokens, 512-token tiles):
  - x is pre-transposed host-side to xT [D, TOK] (bf16) so every
    contraction over D streams straight from HBM into [D-part, token-free]
    tiles -- no on-chip transposes, no PSUM->SBUF copies. Loaded in
    quarter-slabs so the router can start before a full tile lands.
  - x/base/W1/A/B ship as bf16 (halves HBM traffic; verified rel err
    1.13e-2 vs the 2e-2 gate); the output is written bf16 on-device and
    upcast to f32 on the host at gather.
  - Router runs in token-on-free layout; logits are produced token-major
    directly (hs chunk stationary, W2 moving); top-2 softmax via max /
    masked-second-max / sigmoid-ratio / is_ge ops. Normalization (1/sum) is
    deferred into the epilogue.
  - Selected-expert weights are expanded to the stacked expert-rank dim [80]
    with a tiny 0/1 matmul, multiplied into lowT = A_all^T @ xT, and the
    combined output is lowscaled^T @ B_all (B pre-scaled by 2.0 on host),
    fused with  * (1/sum) + base_output  in one DVE op.
  - DMA queues are dedicated: xT on sync (qSP), base prefetch on scalar
    (qAct), out stores on gpsimd (SWDGE) so store waits never block loads.
    Finals for tile t interleave with router t+1 to keep the PE stream free
    of dependency stalls.
"""
import sys

for _p in ("/opt/trn_rl_repo",):
    if _p not in sys.path:
        sys.path.insert(0, _p)

import numpy as np
import ml_dtypes
from contextlib import ExitStack

import concourse.bass as bass
import concourse.tile as tile
from concourse import bacc, mybir
from concourse.bass_utils import run_bass_kernel_spmd

FP = mybir.dt.float32
FR = mybir.dt.float32r
BF = mybir.dt.bfloat16
NEG_BIG = -1e30

N_CORES = 8
B_, S, D = 4, 4096, 2048
E, R, H = 5, 16, 256
SCALING = 32.0 / 16.0
TT = 512
TOK = (B_ * S) // N_CORES


def _build_nc(TOK=TOK, D=D, H=H, E=E, R=R, TT=TT, n_cores=N_CORES):
    from concourse.alu_op_type import AluOpType as A

    NCH = TT // 128
    KD = D // 128
    KH = H // 128
    NT = TOK // TT
    M = E * R
    EP = 8
    ND = D // 512
    NQ = 4            # xT quarter-slabs per tile
    KQ = KD // NQ     # k-blocks per quarter

    assert TOK % TT == 0 and TT % 128 == 0 and D % 512 == 0 and H % 128 == 0

    nc = bacc.Bacc("TRN2", num_devices=n_cores, debug=False)

    xt_d = nc.dram_tensor("xt", [D, TOK], BF, kind="ExternalInput")
    base_d = nc.dram_tensor("base", [TOK, D], BF, kind="ExternalInput")
    a_d = nc.dram_tensor("a_all", [128, KD * M], BF, kind="ExternalInput")
    b_d = nc.dram_tensor("b_all", [M, D], BF, kind="ExternalInput")
    w1_d = nc.dram_tensor("w1", [128, KD * H], BF, kind="ExternalInput")
    b1_d = nc.dram_tensor("b1v", [128, KH], FP, kind="ExternalInput")
    w2_d = nc.dram_tensor("w2", [128, KH * EP], FR, kind="ExternalInput")
    b2b_d = nc.dram_tensor("b2b", [128, NCH * E], FP, kind="ExternalInput")
    e80_d = nc.dram_tensor("e80", [E, M], FR, kind="ExternalInput")
    id_d = nc.dram_tensor("ident", [128, 128], FR, kind="ExternalInput")
    out_d = nc.dram_tensor("out", [TOK, D], BF, kind="ExternalOutput")

    with tile.TileContext(nc) as tc, ExitStack() as ctx:
        const = ctx.enter_context(tc.tile_pool(name="const", bufs=1))
        xtq_pool = ctx.enter_context(tc.tile_pool(name="xtq", bufs=2 * NQ))
        base_pool = ctx.enter_context(tc.tile_pool(name="basep", bufs=8))
        out_pool = ctx.enter_context(tc.tile_pool(name="outp", bufs=6))
        hs_pool = ctx.enter_context(tc.tile_pool(name="hs", bufs=2))
        hst_pool = ctx.enter_context(tc.tile_pool(name="hst", bufs=2))
        sm_pool = ctx.enter_context(tc.tile_pool(name="sm", bufs=2))
        lsc_pool = ctx.enter_context(tc.tile_pool(name="lsc", bufs=2))

        ps_h = ctx.enter_context(tc.tile_pool(name="ps_h", bufs=3, space="PSUM"))
        ps_low = ctx.enter_context(tc.tile_pool(name="ps_low", bufs=1, space="PSUM"))
        ps_out = ctx.enter_context(tc.tile_pool(name="ps_out", bufs=4, space="PSUM"))

        ident = const.tile([128, 128], FR)
        nc.gpsimd.dma_start(ident[:], id_d.ap())
        # All weights at the head of the gpsimd (SWDGE) stream, before any
        # out-stores are queued there.
        w2_sb = const.tile([128, KH, EP], FR)
        nc.gpsimd.dma_start(w2_sb[:], w2_d.ap().rearrange("p (k e) -> p k e", e=EP))
        b1_sb = const.tile([128, KH], FP)
        nc.gpsimd.dma_start(b1_sb[:], b1_d.ap())
        b2b_sb = const.tile([128, NCH, E], FP)
        nc.gpsimd.dma_start(b2b_sb[:], b2b_d.ap().rearrange("p (c e) -> p c e", e=E))
        e80_sb = const.tile([E, M], FR)
        nc.gpsimd.dma_start(e80_sb[:], e80_d.ap())
        w1_sb = const.tile([128, KD, H], BF)
        a_sb = const.tile([128, KD, M], BF)
        bb_sb = const.tile([M, D], BF)
        nc.scalar.dma_start(w1_sb[:], w1_d.ap().rearrange("p (k h) -> p k h", h=H))
        nc.scalar.dma_start(a_sb[:], a_d.ap().rearrange("p (k m) -> p k m", m=M))
        nc.scalar.dma_start(bb_sb[:], b_d.ap())

        def load_xt(t):
            """Load tile t's xT as NQ quarter-slabs [128, KQ, TT]."""
            qs = []
            for q in range(NQ):
                xq = xtq_pool.tile([128, KQ, TT], BF, name="xq")
                nc.sync.dma_start(
                    xq[:],
                    xt_d.ap()[
                        q * KQ * 128 : (q + 1) * KQ * 128,
                        t * TT : (t + 1) * TT,
                    ].rearrange("(k p) n -> p k n", p=128),
                )
                qs.append(xq)
            return qs

        def load_base_chunk(t, c):
            tok0 = t * TT + c * 128
            base_sb = base_pool.tile([128, D], BF, name="base_sb")
            nc.scalar.dma_start(base_sb[:], base_d.ap()[tok0 : tok0 + 128, :])
            return base_sb

        def xt_k(qs, k):
            return qs[k // KQ][:, k % KQ, :]

        def emit_mm1(t, qs):
            h_ps = [
                ps_h.tile([128, 512], FP, tag="hps", name=f"h_ps{h}")
                for h in range(KH)
            ]
            for k in range(KD):
                for h in range(KH):
                    nc.tensor.matmul(
                        h_ps[h][:, 0:TT],
                        w1_sb[:, k, h * 128 : (h + 1) * 128],
                        xt_k(qs, k),
                        start=(k == 0),
                        stop=(k == KD - 1),
                    )
            return h_ps

        def emit_silu(t, h_ps):
            sg_sb = hst_pool.tile([128, KH, TT], FP)
            hs_sb = hs_pool.tile([128, KH, TT], FR)
            for h in range(KH):
                nc.vector.tensor_scalar(
                    hs_sb[:, h, :], h_ps[h][:, 0:TT], b1_sb[:, h : h + 1], None,
                    op0=A.add,
                )
                nc.scalar.activation(
                    sg_sb[:, h, :], h_ps[h][:, 0:TT],
                    mybir.ActivationFunctionType.Sigmoid,
                    bias=b1_sb[:, h : h + 1], scale=1.0,
                )
            nc.gpsimd.tensor_tensor(hs_sb[:], hs_sb[:], sg_sb[:], A.mult)
            return hs_sb

        def emit_low(t, qs):
            low_ps = ps_low.tile([M, 512], FP)
            for k in range(KD):
                nc.tensor.matmul(
                    low_ps[:, 0:TT],
                    a_sb[:, k, :],
                    xt_k(qs, k),
                    start=(k == 0),
                    stop=(k == KD - 1),
                )
            return low_ps

        def emit_mm2_lg(t, hs_sb):
            # token-major logits directly: per chunk, hs slice is the
            # stationary operand, W2 streams (N=8) -- no copy/transpose hops
            lg_ps = ps_h.tile([128, 4, 128], FP, tag="hps")
            for c in range(NCH):
                for k in range(KH):
                    nc.tensor.matmul(
                        lg_ps[:, c, 0:EP],
                        hs_sb[:, k, c * 128 : (c + 1) * 128],
                        w2_sb[:, k, :],
                        start=(k == 0),
                        stop=(k == KH - 1),
                    )
            return lg_ps

        def emit_softmax(t, lg_ps):
            # top-2 softmax, unnormalized (1/sum fused into epilogue)
            Ls = sm_pool.tile([128, NCH, E], FP)
            nc.vector.tensor_tensor(Ls[:], lg_ps[:, 0:NCH, 0:E], b2b_sb[:], A.add)
            nm1 = sm_pool.tile([128, NCH], FP)
            nc.vector.tensor_reduce(
                nm1[:], Ls[:], axis=mybir.AxisListType.X, op=A.max, negate=True
            )
            mk = sm_pool.tile([128, NCH, E], FP)
            eq = sm_pool.tile([128, NCH, E], FP)
            for c in range(NCH):
                nc.vector.tensor_scalar(
                    eq[:, c, :], Ls[:, c, :], nm1[:, c : c + 1], 0.0,
                    op0=A.add, op1=A.is_equal,
                )
                nc.vector.scalar_tensor_tensor(
                    mk[:, c, :], eq[:, c, :], NEG_BIG, Ls[:, c, :],
                    op0=A.mult, op1=A.add,
                )
            nm2 = sm_pool.tile([128, NCH], FP)
            nc.vector.tensor_reduce(
                nm2[:], mk[:], axis=mybir.AxisListType.X, op=A.max, negate=True
            )
            vs = sm_pool.tile([128, NCH, E], FP)
            ve = sm_pool.tile([128, NCH, E], FP)
            om = sm_pool.tile([128, NCH, E], FP)
            ge = sm_pool.tile([128, NCH, E], FP)
            for c in range(NCH):
                nc.scalar.activation(
                    vs[:, c, :], Ls[:, c, :],
                    mybir.ActivationFunctionType.Sigmoid,
                    bias=nm1[:, c : c + 1], scale=1.0,
                )
                nc.vector.tensor_scalar(
                    ge[:, c, :], Ls[:, c, :], nm2[:, c : c + 1], 0.0,
                    op0=A.add, op1=A.is_ge,
                )
            nc.vector.tensor_scalar(
                om[:], vs[:], -1.0, 1.0, op0=A.mult, op1=A.add
            )
            nc.vector.reciprocal(om[:], om[:])
            nc.vector.tensor_tensor(ve[:], vs[:], om[:], A.mult)
            v = sm_pool.tile([128, NCH, E], FR)
            nc.vector.tensor_tensor(v[:], ve[:], ge[:], A.mult)
            s = sm_pool.tile([128, NCH], FP)
            nc.vector.tensor_reduce(s[:], v[:], axis=mybir.AxisListType.X, op=A.add)
            rinv = sm_pool.tile([128, NCH], FP)
            nc.vector.reciprocal(rinv[:], s[:])
            return v, rinv

        def emit_vt_we(t, v):
            vt_ps = ps_h.tile([EP, 512], FR, tag="hps")
            for c in range(NCH):
                nc.tensor.transpose(
                    vt_ps[0:E, c * 128 : (c + 1) * 128], v[:, c, :], ident[:]
                )
            vt_sb = sm_pool.tile([E, TT], FR)
            nc.scalar.copy(vt_sb[:], vt_ps[0:E, 0:TT])
            we_ps = ps_h.tile([M, 512], FP, tag="hps")
            nc.tensor.matmul(
                we_ps[:, 0:TT], e80_sb[:], vt_sb[:], start=True, stop=True
            )
            we_sb = lsc_pool.tile([M, TT], FP)
            nc.scalar.copy(we_sb[:], we_ps[:, 0:TT])
            return we_sb

        def emit_lsc(t, low_ps, we_sb):
            lsc_sb = lsc_pool.tile([M, TT], BF)
            nc.vector.tensor_tensor(lsc_sb[:], low_ps[:, 0:TT], we_sb[:], A.mult)
            return lsc_sb

        def emit_final_chunk(t, c, lsc_sb, rinv, base_sb):
            tok0 = t * TT + c * 128
            o_sb = out_pool.tile([128, D], BF)
            o_pss = []
            for db in range(ND):
                o_ps = ps_out.tile([128, 512], FP, tag="ops", name="o_ps")
                nc.tensor.matmul(
                    o_ps[:],
                    lsc_sb[:, c * 128 : (c + 1) * 128],
                    bb_sb[:, db * 512 : (db + 1) * 512],
                    start=True, stop=True,
                )
                o_pss.append(o_ps)
            for db in range(ND):
                nc.vector.scalar_tensor_tensor(
                    o_sb[:, db * 512 : (db + 1) * 512],
                    o_pss[db][:],
                    rinv[:, c : c + 1],
                    base_sb[:, db * 512 : (db + 1) * 512],
                    op0=A.mult, op1=A.add,
                )
            nc.gpsimd.dma_start(out_d.ap()[tok0 : tok0 + 128, :], o_sb[:])

        # ---- pipeline ----
        xts = {}     # t -> quarter tiles
        bases = {}   # t -> [base chunk tiles]
        lscs = {}    # t -> (lsc_sb, rinv)

        def emit_router(t):
            h_ps = emit_mm1(t, xts[t])
            low_ps = emit_low(t, xts[t])
            hs_sb = emit_silu(t, h_ps)
            lg_ps = emit_mm2_lg(t, hs_sb)
            v, rinv = emit_softmax(t, lg_ps)
            we_sb = emit_vt_we(t, v)
            lscs[t] = (emit_lsc(t, low_ps, we_sb), rinv)
            del xts[t]

        # prologue
        xts[0] = load_xt(0)
        bases[0] = [load_base_chunk(0, c) for c in range(NCH)]
        xts[1] = load_xt(1)
        emit_router(0)
        bases[1] = [load_base_chunk(1, c) for c in range(NCH)]

        for i in range(NT):
            r = i + 1
            if i + 2 < NT:
                xts[i + 2] = load_xt(i + 2)
            if i + 2 < NT:
                bases[i + 2] = [load_base_chunk(i + 2, c) for c in range(NCH)]
            lsc_sb, rinv = lscs.pop(i)
            b_tiles = bases.pop(i)
            # finals(i) interleaved with router(r): out-mm chunks 0,1 first,
            # then the big mm1 stream, then chunks 2,3, then the rest of the
            # router whose PE pieces are dependency-gated.
            emit_final_chunk(i, 0, lsc_sb, rinv, b_tiles[0])
            emit_final_chunk(i, 1, lsc_sb, rinv, b_tiles[1])
            if r < NT:
                h_ps = emit_mm1(r, xts[r])
                hs_sb = emit_silu(r, h_ps)
            emit_final_chunk(i, 2, lsc_sb, rinv, b_tiles[2])
            if r < NT:
                low_ps = emit_low(r, xts[r])
                lg_ps = emit_mm2_lg(r, hs_sb)
            emit_final_chunk(i, 3, lsc_sb, rinv, b_tiles[3])
            if r < NT:
                v, rinv2 = emit_softmax(r, lg_ps)
                we_sb = emit_vt_we(r, v)
                lscs[r] = (emit_lsc(r, low_ps, we_sb), rinv2)
                del xts[r]

    nc.compile()
    return nc


def _host_prep(x, base_output, A, B, W1, b1, W2, b2, n_cores=N_CORES, TT=TT,
               scaling=SCALING):
    Bb, S_, Dd = x.shape
    E_, _, R_ = A.shape
    N = Bb * S_
    TOKc = N // n_cores
    NCH = TT // 128
    xf = np.ascontiguousarray(x.reshape(N, Dd), dtype=np.float32)
    bf = np.ascontiguousarray(base_output.reshape(N, Dd), dtype=np.float32)
    a_all = A.transpose(1, 0, 2).reshape(Dd, E_ * R_)
    a_all = np.ascontiguousarray(
        a_all.reshape(Dd // 128, 128, E_ * R_).transpose(1, 0, 2).reshape(128, -1),
        np.float32)
    b_all = np.ascontiguousarray(B.reshape(E_ * R_, Dd) * scaling, np.float32)
    b2b = np.ascontiguousarray(
        np.broadcast_to(np.tile(np.asarray(b2, np.float32), NCH)[None, :],
                        (128, NCH * E_))
    )
    e80 = np.zeros((E_, E_ * R_), np.float32)
    for e in range(E_):
        e80[e, e * R_ : (e + 1) * R_] = 1.0
    ident = np.eye(128, dtype=np.float32)
    shared = {
        "a_all": a_all.astype(ml_dtypes.bfloat16),
        "b_all": b_all.astype(ml_dtypes.bfloat16),
        "w1": np.ascontiguousarray(
            np.asarray(W1, np.float32).reshape(Dd // 128, 128, -1)
            .transpose(1, 0, 2).reshape(128, -1)).astype(ml_dtypes.bfloat16),
        "b1v": np.ascontiguousarray(
            np.asarray(b1, np.float32).reshape(-1, 128).T),
        "w2": np.ascontiguousarray(
            np.pad(np.asarray(W2, np.float32), ((0, 0), (0, 8 - W2.shape[1])))
            .reshape(-1, 128, 8).transpose(1, 0, 2).reshape(128, -1)),
        "b2b": b2b,
        "e80": e80,
        "ident": ident,
    }
    in_maps = []
    for i in range(n_cores):
        m = dict(shared)
        m["xt"] = np.ascontiguousarray(
            xf[i * TOKc : (i + 1) * TOKc].T.astype(ml_dtypes.bfloat16))
        m["base"] = np.ascontiguousarray(
            bf[i * TOKc : (i + 1) * TOKc].astype(ml_dtypes.bfloat16))
        in_maps.append(m)
    return in_maps, (N, TOKc, Dd)


_NC_CACHE = {}


def _get_nc():
    if "nc" not in _NC_CACHE:
        _NC_CACHE["nc"] = _build_nc()
    return _NC_CACHE["nc"]


def kernel(x, base_output, A, B, W1, b1, W2, b2, _trace=False):
    x = np.asarray(x)
    base_output = np.asarray(base_output)
    nc = _get_nc()
    in_maps, (N, TOKc, Dd) = _host_prep(
        np.asarray(x, np.float32), np.asarray(base_output, np.float32),
        np.asarray(A, np.float32), np.asarray(B, np.float32),
        np.asarray(W1, np.float32), np.asarray(b1, np.float32),
        np.asarray(W2, np.float32), np.asarray(b2, np.float32),
    )
    res = run_bass_kernel_spmd(
        nc, in_maps, core_ids=list(range(N_CORES)), trace=_trace
    )
    out = np.concatenate(
        [np.asarray(res.results[i]["out"]).astype(np.float32) for i in range(N_CORES)],
        axis=0)
    out = out.reshape(x.shape)
    if _trace:
        kernel._last_exec_time_ns = res.exec_time_ns
        kernel._last_results = res
    return out
